# revision 1
# baseline (speedup 1.0000x reference)
"""Deformable bilinear sampling kernel for TRN2 (8-core SPMD).

Algorithm: per (n,o) pair, each output pixel (h,w) needs the 2x2x32c patch at
(h+floor(off_h), w+floor(off_w)) with bilinear corner weights. The host stages
a patch-replicated DRAM tensor P[pair] where row (hh*144+ww) holds the
contiguous 128-float patch at padded anchor (hh,ww); the device computes int16
gather indices + corner weights from the offsets, pulls one 512B row per pixel
with gpsimd.dma_gather, and does a 4-term weighted combine on DVE/GPSIMD.
"""

import numpy as np

import concourse.bacc as bacc
import concourse.bass as bass
import concourse.mybir as mybir
from concourse.library_config import mlp

PAIRS = 4          # (n,o) pairs per core
H = W = 128
C = 32
PAD = 8
HP = 144           # padded anchor grid
NROWS = HP * HP    # 20736 patch rows per pair
NIDX = H * W       # 16384 gathered pixels per pair
CH = 2             # gather chunks per pair
NIDX_CH = NIDX // CH
WCH = W // CH      # w-columns per chunk
NCHUNK = PAIRS * CH

F32 = mybir.dt.float32
I16 = mybir.dt.int16
OP = mybir.AluOpType
TWO23 = 12582912.0  # 1.5 * 2^23: forces round-to-integer in f32 for |x| < 2^22


def build_nc(combine_split=None):
    """combine_split: list of 'v'(vector) or 'g'(gpsimd) per chunk (len 8)."""
    if combine_split is None:
        combine_split = ["v"] * NCHUNK
        combine_split[6] = "g"
        combine_split[7] = "g"
    nc = bacc.Bacc("TRN2")
    patches = nc.declare_dram_parameter("patches", [PAIRS, NROWS, 128], F32, isOutput=False)
    offn = nc.declare_dram_parameter("offn", [PAIRS, 2, H, W], F32, isOutput=False)
    basen = nc.declare_dram_parameter("basen", [H, W], F32, isOutput=False)
    out = nc.declare_dram_parameter("out", [PAIRS, H, W, C], F32, isOutput=True)

    from contextlib import ExitStack

    with ExitStack() as stack:
        ec = stack.enter_context
        block = ec(nc.Block())
        NG = 4   # gather buffers
        NA = 4   # acc buffers
        Gb = [ec(nc.sbuf_tensor(f"G{i}", [128, NIDX_CH // 128, 128], F32)) for i in range(NG)]
        accb = [ec(nc.sbuf_tensor(f"acc{i}", [128, WCH, C], F32)) for i in range(NA)]
        tmpv = ec(nc.sbuf_tensor("tmpv", [128, WCH, C], F32))
        tmpg = ec(nc.sbuf_tensor("tmpg", [128, WCH, C], F32))
        on0 = ec(nc.sbuf_tensor("on0", [128, 2, W], F32))
        on1 = ec(nc.sbuf_tensor("on1", [128, 2, W], F32))
        bnat = ec(nc.sbuf_tensor("bnat", [128, W], F32))
        d0 = ec(nc.sbuf_tensor("d0", [128, 1024], I16))
        d1 = ec(nc.sbuf_tensor("d1", [128, 1024], I16))
        wt0 = ec(nc.sbuf_tensor("wt0", [128, 4, W], F32))
        wt1 = ec(nc.sbuf_tensor("wt1", [128, 4, W], F32))
        sf = ec(nc.sbuf_tensor("sf", [128, 2, W], F32))      # frac (natural)
        sg = ec(nc.sbuf_tensor("sg", [128, 2, W], F32))      # 1-frac (natural)
        sy2 = ec(nc.sbuf_tensor("sy2", [128, 2, W], F32))
        tD = ec(nc.sbuf_tensor("tD", [128, W], F32))
        dnat = ec(nc.sbuf_tensor("dnat", [128, W], I16))
        s_inb = ec(nc.semaphore("s_inb"))    # basew DMA
        s_in0 = ec(nc.semaphore("s_in0"))    # input DMAs for buffer set 0
        s_in1 = ec(nc.semaphore("s_in1"))    # input DMAs for buffer set 1
        s_g = [ec(nc.semaphore(f"s_g{i}")) for i in range(NCHUNK)]  # gather i done
        s_cmb = [ec(nc.semaphore(f"s_cmb{i}")) for i in range(NCHUNK)]  # combine i done
        s_out = [ec(nc.semaphore(f"s_out{i}")) for i in range(NCHUNK)]  # out i done
        s_inx = [s_in0, s_in1]
        s_dn = ec(nc.semaphore("s_dn"))      # dnat ready (inc 1/pair)
        s_wt = ec(nc.semaphore("s_wt"))      # weights ready (inc 1/pair)
        s_dw = ec(nc.semaphore("s_dw"))      # ACT wrap copies (inc 1 each, 16/pair)
        s_cv = ec(nc.semaphore("s_cv"))      # vector same-engine chain
        s_cg = ec(nc.semaphore("s_cg"))      # gpsimd same-engine chain
        onb = [on0, on1]
        db = [d0, d1]
        wtb = [wt0, wt1]


        @block.sync
        def _(sync: bass.BassEngine):
            sync.dma_start(bnat[:, :], basen[:, :]).then_inc(s_inb, 16)
            for p in range(min(2, PAIRS)):
                sync.dma_start(onb[p % 2][:, :, :], offn[p, :, :, :].transpose([1, 0, 2])).then_inc(s_inx[p % 2], 16)
            for s in range(NCHUNK):
                p, c = divmod(s, CH)
                if c == 0 and p + 2 < PAIRS:
                    # refill input tile of set p%2: vector must be done with
                    # pair p's idx/weights math (it reads on[st]).
                    sync.wait_ge(s_dn, p + 1)
                    pp = p + 2
                    sync.dma_start(onb[pp % 2][:, :, :], offn[pp, :, :, :].transpose([1, 0, 2])).then_inc(s_inx[pp % 2], 16)
                # out DMA for chunk s
                sync.wait_ge(s_cmb[s], 1)
                dst = out[p, :, c * WCH:(c + 1) * WCH, :]   # (h, w, c)
                sync.dma_start(dst, accb[s % NA][:, :, :]).then_inc(s_out[s], 16)

        class Chain:
            """Serializes dependent ops on one engine via a chain semaphore:
            wait for all previously-registered ops, then run the thunk and
            register its instruction."""

            def __init__(self, eng, sem):
                self.eng, self.sem, self.n = eng, sem, 0
                self.extra = []

            def run(self, thunk, final=None):
                # final=(sem, value_after): inc that sem instead of the chain
                if self.n:
                    self.eng.wait_ge(self.sem, self.n)
                for sem, val in self.extra:
                    self.eng.wait_ge(sem, val)
                self.extra = []
                inst = thunk()
                if final is None:
                    inst.then_inc(self.sem, 1)
                    self.n += 1
                else:
                    sem, val = final
                    inst.then_inc(sem, 1)
                    self.extra.append((sem, val))
                return inst

        def idx_weights(eng, ch, p):
            st = p % 2
            onf = onb[st][:, :, :]      # [128, 2, W] natural offsets
            r = ch.run
            wt = wtb[st]
            # ---- floors/fracs (natural layout, shared) ----
            r(lambda: eng.tensor_scalar(sy2[:, :, :], onf, TWO23, -TWO23, OP.add, OP.add))
            r(lambda: eng.tensor_tensor(sf[:, :, :], sy2[:, :, :], onf, OP.is_gt))
            r(lambda: eng.tensor_sub(sy2[:, :, :], sy2[:, :, :], sf[:, :, :]))   # floors
            r(lambda: eng.tensor_sub(sf[:, :, :], onf, sy2[:, :, :]))            # frac
            # ---- gather indices first (unblocks ACT + the gather DMA asap) ----
            r(lambda: eng.scalar_tensor_tensor(tD[:, :], sy2[:, 0, :], float(HP), sy2[:, 1, :], OP.mult, OP.add))
            r(lambda: eng.tensor_add(tD[:, :], tD[:, :], bnat[:, :]))
            r(lambda: eng.tensor_copy(dnat[:, :], tD[:, :]), final=(s_dn, p + 1))
            # ---- weights ----
            r(lambda: eng.tensor_scalar(sg[:, :, :], sf[:, :, :], -1.0, 1.0, OP.mult, OP.add))
            r(lambda: eng.tensor_mul(wt[:, 0, :], sg[:, 0, :], sg[:, 1, :]))
            r(lambda: eng.tensor_mul(wt[:, 1, :], sg[:, 0, :], sf[:, 1, :]))
            r(lambda: eng.tensor_mul(wt[:, 2, :], sf[:, 0, :], sg[:, 1, :]))
            r(lambda: eng.tensor_mul(wt[:, 3, :], sf[:, 0, :], sf[:, 1, :]), final=(s_wt, p + 1))
            return ch

        def emit_combine(eng, ch, s, tmp):
            p, c = divmod(s, CH)
            st = p % 2
            G = Gb[s % NG]
            acc = accb[s % NA]
            gflat = G[:, :, :]  # [128, WCH, 128]; slot k = cols k*C:(k+1)*C
            wt = wtb[st]
            ws = c * WCH
            r = ch.run

            def gk(k):
                return gflat[:, :, k * C:(k + 1) * C]

            def wk(k):
                a = wt[:, k, ws:ws + WCH]          # [128, WCH]
                return a.unsqueeze(2).broadcast_to([128, WCH, C])

            r(lambda: eng.tensor_mul(acc[:, :, :], gk(0), wk(0)))
            r(lambda: eng.tensor_mul(tmp[:, :, :], gk(1), wk(1)))
            r(lambda: eng.tensor_add(acc[:, :, :], acc[:, :, :], tmp[:, :, :]))
            r(lambda: eng.tensor_mul(tmp[:, :, :], gk(2), wk(2)))
            r(lambda: eng.tensor_add(acc[:, :, :], acc[:, :, :], tmp[:, :, :]))
            r(lambda: eng.tensor_mul(tmp[:, :, :], gk(3), wk(3)))
            return lambda final: r(
                lambda: eng.tensor_add(acc[:, :, :], acc[:, :, :], tmp[:, :, :]),
                final=final,
            )

        @block.vector
        def _(vector: bass.BassEngine):
            ch = Chain(vector, s_cv)
            # one-time: zero the wrapped-idx tiles (the gather AP spans all 128
            # partitions; only 0-31 carry real data)
            ch.run(lambda: vector.memset(d0[:, :], 0))
            ch.run(lambda: vector.memset(d1[:, :], 0))
            vector.wait_ge(s_inb, 16)
            for p in range(min(2, PAIRS)):
                vector.wait_ge(s_inx[p % 2], 16)
                if p >= 1:
                    # dnat reuse: ACT wrap-copies of pair p-1 must be done
                    vector.wait_ge(s_dw, 84 * p - 16)
                idx_weights(vector, ch, p)
            for s in range(NCHUNK):
                p, c = divmod(s, CH)
                if combine_split[s] == "v":
                    vector.wait_ge(s_g[s], 16)
                    if s >= NA:
                        vector.wait_ge(s_out[s - NA], 16)
                    emit_combine(vector, ch, s, tmpv)((s_cmb[s], 1))
                if c == CH - 1 and p + 2 < PAIRS:
                    pp = p + 2
                    vector.wait_ge(s_inx[pp % 2], 16 * (pp // 2 + 1))
                    # dnat reuse: ACT wrap-copies of pair pp-1 must be done
                    vector.wait_ge(s_dw, 84 * pp - 16)
                    # wt[p%2] reuse: combines of pair p must be done
                    vector.wait_ge(s_cmb[CH * p], 1)
                    vector.wait_ge(s_cmb[CH * p + 1], 1)
                    idx_weights(vector, ch, pp)

        @block.scalar
        def _(act: bass.BassEngine):
            # rearrange dnat [128h, 128w] -> wrapped d[st] partitions 0-31:
            # d[g*16+q, w*8+k] = dnat[q+16k, w]  (g = replication group)
            for p in range(PAIRS):
                st = p % 2
                if p >= 1:
                    act.wait_ge(s_dw, 84 * p)   # drain own prior-pair DMA incs
                act.wait_ge(s_dn, p + 1)
                if p >= 2:
                    # d[st] reuse: gathers of pair p-2 must be done
                    act.wait_ge(s_g[CH * (p - 2)], 16)
                    act.wait_ge(s_g[CH * (p - 2) + 1], 16)
                dwrap = db[st][:, :].rearrange("p (w k) -> p w k", k=8)
                for k in range(0, 8, 2):   # even k: engine copy (32-aligned src)
                    act.copy(dwrap[0:16, :, k],
                             dnat[16 * k:16 * (k + 1), :]).then_inc(s_dw, 1)
                with nc.allow_non_contiguous_dma(reason="4KB idx-wrap strided dst"):
                    for k in range(1, 8, 2):   # odd k: tiny DMA (no partition align)
                        act.dma_start(dwrap[0:16, :, k],
                                      dnat[16 * k:16 * (k + 1), :]).then_inc(s_dw, 16)
                # engine copies + DMAs above: 4*1 + 4*16 = 68 incs per pair
                act.wait_ge(s_dw, 84 * p + 68)
                # replicate wrapped indices to partitions 16-31 (the group the
                # Q7 descriptor-gen core actually reads on HW)
                act.dma_start(db[st][16:32, :], db[st][0:16, :]).then_inc(s_dw, 16)

        @block.gpsimd
        def _(gpsimd: bass.BassGpSimd):
            chg = Chain(gpsimd, s_cg)
            gpsimd.load_library(mlp)
            for s in range(NCHUNK):
                p, c = divmod(s, CH)
                gpsimd.wait_ge(s_dw, 84 * (p + 1))
                if s >= NG:
                    gpsimd.wait_ge(s_cmb[s - NG], 1)  # G[s%NG] free
                gpsimd.dma_gather(
                    Gb[s % NG][:, :, :],
                    patches[p, :, :],
                    db[p % 2][:, c * 512:(c + 1) * 512],
                    NIDX_CH,
                    NIDX_CH,
                    128,
                    single_packet=False,
                ).then_inc(s_g[s], 16)
            for s in range(NCHUNK):
                if combine_split[s] == "g":
                    gpsimd.wait_ge(s_wt, s // CH + 1)
                    gpsimd.wait_ge(s_g[s], 16)
                    if s >= NA:
                        gpsimd.wait_ge(s_out[s - NA], 16)
                    emit_combine(gpsimd, chg, s, tmpg)((s_cmb[s], 1))

    nc.compile()
    return nc


# ---------------- host-side helpers ----------------

def build_patches_all(imgs_pairs):
    """imgs_pairs: (NPAIR, C, H, W) f32 -> (NPAIR, NROWS, 128) f32"""
    npair = imgs_pairs.shape[0]
    hw_c = np.ascontiguousarray(np.transpose(imgs_pairs, (0, 2, 3, 1)))  # (P,H,W,C)
    padded = np.zeros((npair, HP + 1, HP + 1, C), np.float32)
    padded[:, PAD:PAD + H, PAD:PAD + W] = hw_c
    P = np.empty((npair, HP, HP, 4, C), np.float32)
    P[:, :, :, 0] = padded[:, 0:HP, 0:HP]
    P[:, :, :, 1] = padded[:, 0:HP, 1:HP + 1]
    P[:, :, :, 2] = padded[:, 1:HP + 1, 0:HP]
    P[:, :, :, 3] = padded[:, 1:HP + 1, 1:HP + 1]
    return P.reshape(npair, NROWS, 128)


def base_natural():
    h = np.arange(H).reshape(H, 1)
    w = np.arange(W).reshape(1, W)
    return ((h + PAD) * HP + (w + PAD)).astype(np.float32)


def make_in_map(imgs_pairs, offp):
    return {
        "patches": build_patches_all(imgs_pairs),
        "offn": np.ascontiguousarray(offp),
        "basen": base_natural(),
    }


# ---------------- public entry point ----------------

N_CORES = 8
PAIRS_TOTAL = 32

LAST_EXEC_TIME_NS = None


def kernel(images, offsets):
    """images (4,8,32,128,128) f32; offsets (4,16,128,128) f32 ->
    (4,8,32,128,128) f32 deformable bilinear sampling, on 8 NeuronCores."""
    import os
    global LAST_EXEC_TIME_NS
    from concourse.bass_utils import run_bass_kernel_spmd

    images = np.ascontiguousarray(np.asarray(images, dtype=np.float32))
    offsets = np.ascontiguousarray(np.asarray(offsets, dtype=np.float32))
    imgs = images.reshape(PAIRS_TOTAL, C, H, W)
    offp = offsets.reshape(4, 8, 2, H, W).reshape(PAIRS_TOTAL, 2, H, W)

    nc = build_nc()
    in_maps = []
    for core in range(N_CORES):
        sl = slice(core * PAIRS, (core + 1) * PAIRS)
        in_maps.append(make_in_map(imgs[sl], offp[sl]))
    trace = bool(os.environ.get("DK_TRACE"))
    res = run_bass_kernel_spmd(nc, in_maps, list(range(N_CORES)), trace=trace)
    if trace:
        LAST_EXEC_TIME_NS = res.exec_time_ns
        if res.instructions_and_trace:
            print("trace path:", res.instructions_and_trace[1])
    outs = [np.asarray(res.results[i]["out"]) for i in range(N_CORES)]
    full = np.concatenate(outs, axis=0)            # (32, H, W, C)
    full = np.transpose(full, (0, 3, 1, 2))        # (32, C, H, W)
    return np.ascontiguousarray(full.reshape(4, 8, C, H, W)).astype(np.float32)



# revision 7
# speedup vs baseline: 2.0096x; 2.0096x over previous
"""Deformable bilinear sampling kernel for TRN2 (8-core SPMD), v2.

Per (n,o) pair, each output pixel (h,w) needs the 2x2xC patch at
(h+floor(off_h), w+floor(off_w)) with bilinear corner weights. The host stages
a patch tensor P[pair] where row (hh*144+ww) holds the 256B fp16 patch at
padded anchor (hh,ww), laid out c-major with the 4 corners packed per channel
(so one u64 = one channel's 4 corners). The device computes int16 gather
indices on DVE + corner-weight products, pulls one 256B row per pixel with
gpsimd.dma_gather (u32-aliased: 64 "elements"/row), then combines with a
single 2x-mode fused multiply (k packed last) + two tree adds, and writes
fp16 output.

Engine split: Pool = fracs math, gathers, fold2 (+some fold1); DVE = idx math,
weight products, muls, fold1; ACT = idx wrap; SP = all input/output DMAs.
"""

import numpy as np

import concourse.bacc as bacc
import concourse.bass as bass
import concourse.mybir as mybir
from concourse.library_config import mlp

import os as _os

PAIRS = 4          # (n,o) pairs per core
H = W = 128
C = 32
PAD = 8
HP = 144           # padded anchor grid
NROWS = HP * HP    # 20736 patch rows per pair
NIDX = H * W       # 16384 gathered pixels per pair
CH = int(_os.environ.get("V2_CH", "4"))   # gather chunks per pair
NIDX_CH = NIDX // CH
WCH = W // CH      # w-columns per chunk
NCHUNK = PAIRS * CH
NG = int(_os.environ.get("V2_NG", "6"))   # gather buffers
NP_ = int(_os.environ.get("V2_NP", "3"))  # product buffers
NA = int(_os.environ.get("V2_NA", "3"))   # fold1 buffers
NB = int(_os.environ.get("V2_NB", "6"))   # out buffers

F32 = mybir.dt.float32
F16 = mybir.dt.float16
U64 = mybir.dt.uint64
I16 = mybir.dt.int16
OP = mybir.AluOpType
TWO23 = 12582912.0  # 1.5 * 2^23: forces round-to-integer in f32 for |x| < 2^22


def build_nc(fold1_split=None):
    """fold1_split: list of 'v'(vector) or 'g'(gpsimd) per chunk (len 8)."""
    if fold1_split is None:
        env = _os.environ.get("V2_SPLIT")
        if env:
            fold1_split = list(env)
        elif NCHUNK == 16:
            fold1_split = list("gvvgvvvgvvgvvvgv")
        else:
            fold1_split = ["v", "g"] * (NCHUNK // 2)
    assert len(fold1_split) == NCHUNK
    nc = bacc.Bacc("TRN2")
    # u32-declared (JAX canonicalizes u64 params); gathered as a u64 view
    patches = nc.declare_dram_parameter("patches", [PAIRS, NROWS, 64], mybir.dt.uint32, isOutput=False)
    offn = nc.declare_dram_parameter("offn", [PAIRS, 2, H, W], F32, isOutput=False)
    basen = nc.declare_dram_parameter("basen", [H, W], F32, isOutput=False)
    out = nc.declare_dram_parameter("out", [PAIRS, H, W, C], F16, isOutput=True)

    from contextlib import ExitStack

    with ExitStack() as stack:
        ec = stack.enter_context
        block = ec(nc.Block())
        Gb = [ec(nc.sbuf_tensor(f"G{i}", [128, WCH, 64], mybir.dt.uint32)) for i in range(NG)]
        Pb = [ec(nc.sbuf_tensor(f"P{i}", [128, WCH, C, 4], F16)) for i in range(NP_)]
        Ab = [ec(nc.sbuf_tensor(f"A{i}", [128, WCH, C, 2], F16)) for i in range(NA)]
        Bb = [ec(nc.sbuf_tensor(f"B{i}", [128, WCH, C], F16)) for i in range(NB)]
        on0 = ec(nc.sbuf_tensor("on0", [128, 2, W], F32))
        on1 = ec(nc.sbuf_tensor("on1", [128, 2, W], F32))
        onb = [on0, on1]
        bnat = ec(nc.sbuf_tensor("bnat", [128, W], F32))
        sy2b = [ec(nc.sbuf_tensor(f"sy2_{i}", [128, 2, W], F32)) for i in range(2)]
        sfb = [ec(nc.sbuf_tensor(f"sf{i}", [128, 2, W], F32)) for i in range(2)]
        sgb = [ec(nc.sbuf_tensor(f"sg{i}", [128, 2, W], F32)) for i in range(2)]
        wtkb = [ec(nc.sbuf_tensor(f"wtk{i}", [128, W, 4], F16)) for i in range(2)]
        tD = ec(nc.sbuf_tensor("tD", [128, W], F32))
        dnatb = [ec(nc.sbuf_tensor(f"dnat{i}", [128, W], I16)) for i in range(2)]
        d0 = ec(nc.sbuf_tensor("d0", [128, 1024], I16))
        d1 = ec(nc.sbuf_tensor("d1", [128, 1024], I16))
        db = [d0, d1]

        s_inb = ec(nc.semaphore("s_inb"))    # basen DMA
        s_in0 = ec(nc.semaphore("s_in0"))    # offn DMAs buffer 0
        s_in1 = ec(nc.semaphore("s_in1"))    # offn DMAs buffer 1
        s_inx = [s_in0, s_in1]
        s_frac = ec(nc.semaphore("s_frac"))  # Pool fracs done (1/pair)
        s_ix = ec(nc.semaphore("s_ix"))      # DVE done reading sf/sg/sy2 (1/pair)
        s_dn = ec(nc.semaphore("s_dn"))      # dnat ready (1/pair)
        s_wr = ec(nc.semaphore("s_wr"))      # ACT wrap progress (68/pair)
        s_rp = ec(nc.semaphore("s_rp"))      # ACT replica DMAs (16/pair)
        s_g = [ec(nc.semaphore(f"s_g{i}")) for i in range(NCHUNK)]    # gather done (16)
        s_m = [ec(nc.semaphore(f"s_m{i}")) for i in range(NCHUNK)]    # mul done (1)
        s_f1 = [ec(nc.semaphore(f"s_f1_{i}")) for i in range(NCHUNK)]  # fold1 done (1)
        s_f2 = [ec(nc.semaphore(f"s_f2_{i}")) for i in range(NCHUNK)]  # fold2 done (1)
        s_out = [ec(nc.semaphore(f"s_out{i}")) for i in range(NCHUNK)]  # out DMA done (16)
        s_cv = ec(nc.semaphore("s_cv"))      # DVE same-engine chain
        s_cg = ec(nc.semaphore("s_cg"))      # Pool same-engine chain

        class Chain:
            """Serializes dependent ops on one engine via a chain semaphore."""

            def __init__(self, eng, sem):
                self.eng, self.sem, self.n = eng, sem, 0
                self.extra = []

            def run(self, thunk, final=None):
                if self.n:
                    self.eng.wait_ge(self.sem, self.n)
                for sem, val in self.extra:
                    self.eng.wait_ge(sem, val)
                self.extra = []
                inst = thunk()
                if final is None:
                    inst.then_inc(self.sem, 1)
                    self.n += 1
                else:
                    sem, val = final
                    inst.then_inc(sem, 1)
                    self.extra.append((sem, val))
                return inst

        @block.sync
        def _(sync: bass.BassEngine):
            sync.dma_start(onb[0][:, :, :], offn[0, :, :, :].transpose([1, 0, 2])).then_inc(s_inx[0], 16)
            sync.dma_start(bnat[:, :], basen[:, :]).then_inc(s_inb, 16)
            sync.dma_start(onb[1][:, :, :], offn[1, :, :, :].transpose([1, 0, 2])).then_inc(s_inx[1], 16)
            # eager refills: onb[p%2] for pair p+2 once Pool's fracs(p) is done
            for p in range(PAIRS - 2):
                sync.wait_ge(s_frac, p + 1)
                pp = p + 2
                sync.dma_start(onb[pp % 2][:, :, :], offn[pp, :, :, :].transpose([1, 0, 2])).then_inc(s_inx[pp % 2], 16)
            for s in range(NCHUNK):
                p, c = divmod(s, CH)
                sync.wait_ge(s_f2[s], 1)
                dst = out[p, :, c * WCH:(c + 1) * WCH, :]   # (h, w, c)
                sync.dma_start(dst, Bb[s % NB][:, :, :]).then_inc(s_out[s], 16)

        def emit_idx_weights(ch, p):
            """DVE: gather indices dnat + fp16 weight products wtk for pair p."""
            eng = ch.eng
            st = p % 2
            sy2, sf, sg = sy2b[st], sfb[st], sgb[st]
            wtk = wtkb[st]
            dnat = dnatb[st]
            r = ch.run
            # idx first (unblocks ACT wrap asap)
            r(lambda: eng.scalar_tensor_tensor(tD[:, :], sy2[:, 0, :], float(HP), sy2[:, 1, :], OP.mult, OP.add))
            r(lambda: eng.tensor_add(dnat[:, :], tD[:, :], bnat[:, :]), final=(s_dn, p + 1))
            # weight products -> wtk[:, :, k], k order (00, 01, 10, 11)
            r(lambda: eng.tensor_mul(wtk[:, :, 0], sg[:, 0, :], sg[:, 1, :]))
            r(lambda: eng.tensor_mul(wtk[:, :, 1], sg[:, 0, :], sf[:, 1, :]))
            r(lambda: eng.tensor_mul(wtk[:, :, 2], sf[:, 0, :], sg[:, 1, :]))
            r(lambda: eng.tensor_mul(wtk[:, :, 3], sf[:, 0, :], sf[:, 1, :]), final=(s_ix, p + 1))

        def emit_fold1(ch, s):
            P = Pb[s % NP_][:, :, :, :]
            A = Ab[s % NA]
            return ch.run(lambda: ch.eng.tensor_add(A[:, :, :, :], P[:, :, :, 0:2], P[:, :, :, 2:4]),
                          final=(s_f1[s], 1))

        @block.vector
        def _(vector: bass.BassEngine):
            ch = Chain(vector, s_cv)
            # zero the wrapped-idx tiles once
            ch.run(lambda: vector.memset(d0[:, :].bitcast(mybir.dt.uint32), 0))
            ch.run(lambda: vector.memset(d1[:, :].bitcast(mybir.dt.uint32), 0))
            vector.wait_ge(s_inb, 16)
            for p in range(2):
                vector.wait_ge(s_frac, p + 1)
                emit_idx_weights(ch, p)
            for s in range(NCHUNK):
                p, c = divmod(s, CH)
                # mul: P[s%2] = G4 * W4
                vector.wait_ge(s_g[s], 16)
                if s >= NP_ and fold1_split[s - NP_] == "g":
                    vector.wait_ge(s_f1[s - NP_], 1)   # P[s%NP_] free
                st = p % 2
                G4 = Gb[s % NG][:, :, :].bitcast(F16).rearrange("p w (c k) -> p w c k", k=4)
                W4 = wtkb[st][:, c * WCH:(c + 1) * WCH, None, :].broadcast_to([128, WCH, C, 4])
                P = Pb[s % NP_]
                ch.run(lambda G4=G4, W4=W4, P=P: vector.tensor_mul(P[:, :, :, :], G4, W4),
                       final=(s_m[s], 1))
                if fold1_split[s] == "v":
                    if s >= NA:
                        vector.wait_ge(s_f2[s - NA], 1)   # A[s%NA] free
                    emit_fold1(ch, s)
                if c == CH - 1 and p + 2 < PAIRS:
                    pp = p + 2
                    vector.wait_ge(s_frac, pp + 1)
                    # dnat[pp%2] reuse: ACT wrap of pair pp-2 complete
                    vector.wait_ge(s_wr, 84 * (pp - 1))
                    emit_idx_weights(ch, pp)

        @block.scalar
        def _(act: bass.BassEngine):
            # rearrange dnat [128h, 128w] -> wrapped d[st] partitions 0-31:
            # d[q, w*8+k] = dnat[q+16k, w]
            for p in range(PAIRS):
                st = p % 2
                if p >= 1:
                    act.wait_ge(s_wr, 84 * p)   # drain own prior-pair DMA incs
                act.wait_ge(s_dn, p + 1)
                if p >= 2:
                    # d[st] reuse: gathers of pair p-2 must be done
                    act.wait_ge(s_g[CH * (p - 2)], 16)
                    act.wait_ge(s_g[CH * (p - 2) + 1], 16)
                dnat = dnatb[st]
                dwrap = db[st][:, :].rearrange("p (w k) -> p w k", k=8)
                for k in range(0, 8, 2):   # even k: engine copy (32-aligned src)
                    act.copy(dwrap[0:16, :, k],
                             dnat[16 * k:16 * (k + 1), :]).then_inc(s_wr, 1)
                with nc.allow_non_contiguous_dma(reason="4KB idx-wrap strided dst"):
                    for k in range(1, 8, 2):   # odd k: tiny DMA (no partition align)
                        act.dma_start(dwrap[0:16, :, k],
                                      dnat[16 * k:16 * (k + 1), :]).then_inc(s_wr, 16)
                act.wait_ge(s_wr, 84 * p + 68)
                # replicate wrapped indices to partitions 16-31 (HW Q7 cores
                # read that group; the simulator reads 0-15)
                act.dma_start(db[st][16:32, :], db[st][0:16, :]).then_inc(s_wr, 16)

        @block.gpsimd
        def _(gpsimd: bass.BassGpSimd):
            chg = Chain(gpsimd, s_cg)
            gpsimd.load_library(mlp)

            def frac_final_fix(p):
                # emit_fracs used final=(s_frac, 0) marker; replace with actual
                pass

            def emit_fracs_pool(p):
                st = p % 2
                gpsimd.wait_ge(s_inx[st], 16 * (p // 2 + 1))
                if p >= 2:
                    gpsimd.wait_ge(s_ix, p - 1)   # DVE done with sf/sg/sy2[st]
                onf = onb[st][:, :, :]
                sy2, sf, sg = sy2b[st], sfb[st], sgb[st]
                r = chg.run
                # floor(x) = round_ne(x-0.5) via the fp32 TWO23 trick.
                # Ties (x within ~1ulp of an integer) may floor one down, but
                # bilinear interpolation is continuous there: the weight
                # compensates the index exactly, so the output is unchanged.
                r(lambda: gpsimd.tensor_scalar(sy2[:, :, :], onf, -0.5, TWO23, OP.add, OP.add))
                r(lambda: gpsimd.tensor_scalar(sy2[:, :, :], sy2[:, :, :], -TWO23, 0.0, OP.add, OP.add))
                r(lambda: gpsimd.tensor_sub(sf[:, :, :], onf, sy2[:, :, :]))
                r(lambda: gpsimd.tensor_scalar(sg[:, :, :], sf[:, :, :], -1.0, 1.0, OP.mult, OP.add),
                  final=(s_frac, p + 1))

            def emit_gather(s):
                p, c = divmod(s, CH)
                gpsimd.wait_ge(s_wr, 84 * (p + 1))
                if s >= NG:
                    gpsimd.wait_ge(s_m[s - NG], 1)   # G[s%NG] free
                ic = NIDX_CH // 16
                gpsimd.dma_gather(
                    Gb[s % NG][:, :, :],
                    patches[p, :, :],
                    db[p % 2][:, c * ic:(c + 1) * ic],
                    NIDX_CH,
                    NIDX_CH,
                    64,
                    single_packet=False,
                ).then_inc(s_g[s], 16)

            def emit_fold2(s):
                A = Ab[s % NA]
                B = Bb[s % NB]
                gpsimd.wait_ge(s_f1[s], 1)
                if s >= NB:
                    gpsimd.wait_ge(s_out[s - NB], 16)   # B[s%NB] free
                chg.run(lambda: gpsimd.tensor_add(B[:, :, :], A[:, :, :, 0], A[:, :, :, 1]),
                        final=(s_f2[s], 1))

            def emit_fold1_pool(s):
                gpsimd.wait_ge(s_m[s], 1)
                if s >= NA:
                    gpsimd.wait_ge(s_f2[s - NA], 1)   # A[s%NA] free
                emit_fold1(chg, s)

            # static schedule: fracs interleaved with gathers and folds
            order_env = _os.environ.get("V2_POOL_ORDER")
            lag = int(_os.environ.get("V2_LAG", "2"))
            if order_env:
                order = [tuple(tok.split(":")) for tok in order_env.split(",")]
                order = [(a, int(b)) for a, b in order]
            else:
                order = [("fr", 0), ("fr", 1)]
                done = 0
                for s in range(NCHUNK):
                    p, c = divmod(s, CH)
                    if c == 1 and p + 2 < PAIRS:
                        order.append(("fr", p + 2))
                    order.append(("g", s))
                    while done <= s - lag:
                        order.append(("f1", done))
                        order.append(("f2", done))
                        done += 1
                while done < NCHUNK:
                    order.append(("f1", done))
                    order.append(("f2", done))
                    done += 1
            for kind, i in order:
                if kind == "fr":
                    emit_fracs_pool(i)
                elif kind == "g":
                    emit_gather(i)
                elif kind == "f1":
                    if fold1_split[i] == "g":
                        emit_fold1_pool(i)
                elif kind == "f2":
                    emit_fold2(i)

    nc.compile()
    return nc


# ---------------- host-side helpers ----------------

def build_patches_all(imgs_pairs):
    """imgs_pairs: (NPAIR, C, H, W) f32 -> (NPAIR, NROWS, 32) u64.

    Row at anchor (hh, ww) = fp16[c][k]: c-major, 4 corners packed per
    channel: k order (0,0), (0,1), (1,0), (1,1)."""
    npair = imgs_pairs.shape[0]
    hw_c = np.ascontiguousarray(np.transpose(imgs_pairs, (0, 2, 3, 1))).astype(np.float16)
    padded = np.zeros((npair, HP + 1, HP + 1, C), np.float16)
    padded[:, PAD:PAD + H, PAD:PAD + W] = hw_c
    P = np.empty((npair, HP, HP, C, 4), np.float16)
    P[:, :, :, :, 0] = padded[:, 0:HP, 0:HP]
    P[:, :, :, :, 1] = padded[:, 0:HP, 1:HP + 1]
    P[:, :, :, :, 2] = padded[:, 1:HP + 1, 0:HP]
    P[:, :, :, :, 3] = padded[:, 1:HP + 1, 1:HP + 1]
    return np.ascontiguousarray(P).reshape(npair, NROWS, 128).view(np.uint32)


def base_natural():
    h = np.arange(H).reshape(H, 1)
    w = np.arange(W).reshape(1, W)
    return ((h + PAD) * HP + (w + PAD)).astype(np.float32)


def make_in_map(imgs_pairs, offp):
    return {
        "patches": build_patches_all(imgs_pairs),
        "offn": np.ascontiguousarray(offp),
        "basen": base_natural(),
    }


# ---------------- public entry point ----------------

N_CORES = 8
PAIRS_TOTAL = 32

LAST_EXEC_TIME_NS = None


def kernel(images, offsets):
    """images (4,8,32,128,128) f32; offsets (4,16,128,128) f32 ->
    (4,8,32,128,128) f32 deformable bilinear sampling, on 8 NeuronCores."""
    import os
    global LAST_EXEC_TIME_NS
    from concourse.bass_utils import run_bass_kernel_spmd

    images = np.ascontiguousarray(np.asarray(images, dtype=np.float32))
    offsets = np.ascontiguousarray(np.asarray(offsets, dtype=np.float32))
    imgs = images.reshape(PAIRS_TOTAL, C, H, W)
    offp = offsets.reshape(4, 8, 2, H, W).reshape(PAIRS_TOTAL, 2, H, W)

    nc = build_nc()
    in_maps = []
    for core in range(N_CORES):
        sl = slice(core * PAIRS, (core + 1) * PAIRS)
        in_maps.append(make_in_map(imgs[sl], offp[sl]))
    trace = bool(os.environ.get("DK_TRACE"))
    res = run_bass_kernel_spmd(nc, in_maps, list(range(N_CORES)), trace=trace)
    if trace:
        LAST_EXEC_TIME_NS = res.exec_time_ns
        if res.instructions_and_trace:
            print("trace path:", res.instructions_and_trace[1])
    outs = [np.asarray(res.results[i]["out"]) for i in range(N_CORES)]
    full = np.concatenate(outs, axis=0).astype(np.float32)   # (32, H, W, C)
    full = np.transpose(full, (0, 3, 1, 2))                  # (32, C, H, W)
    return np.ascontiguousarray(full.reshape(4, 8, C, H, W)).astype(np.float32)


# revision 8
# speedup vs baseline: 2.0979x; 1.0439x over previous
"""Deformable bilinear sampling kernel for TRN2 (8-core SPMD), v2.

Per (n,o) pair, each output pixel (h,w) needs the 2x2xC patch at
(h+floor(off_h), w+floor(off_w)) with bilinear corner weights. The host stages
a patch tensor P[pair] where row (hh*144+ww) holds the 256B fp16 patch at
padded anchor (hh,ww), laid out c-major with the 4 corners packed per channel
(so one u64 = one channel's 4 corners). The device computes int16 gather
indices on DVE + corner-weight products, pulls one 256B row per pixel with
gpsimd.dma_gather (u32-aliased: 64 "elements"/row), then combines with a
single 2x-mode fused multiply (k packed last) + two tree adds, and writes
fp16 output.

Engine split: Pool = fracs math, gathers, fold2 (+some fold1); DVE = idx math,
weight products, muls, fold1; ACT = idx wrap; SP = all input/output DMAs.
"""

import numpy as np

import concourse.bacc as bacc
import concourse.bass as bass
import concourse.mybir as mybir
from concourse.library_config import mlp

import os as _os

PAIRS = 4          # (n,o) pairs per core
H = W = 128
C = 32
PAD = 8
HP = 144           # padded anchor grid
NROWS = HP * HP    # 20736 patch rows per pair
NIDX = H * W       # 16384 gathered pixels per pair
CH = int(_os.environ.get("V2_CH", "4"))   # gather chunks per pair
NIDX_CH = NIDX // CH
WCH = W // CH      # w-columns per chunk
NCHUNK = PAIRS * CH
NG = int(_os.environ.get("V2_NG", "6"))   # gather buffers
NP_ = int(_os.environ.get("V2_NP", "3"))  # product buffers
NA = int(_os.environ.get("V2_NA", "3"))   # fold1 buffers
NB = int(_os.environ.get("V2_NB", "6"))   # out buffers

F32 = mybir.dt.float32
F16 = mybir.dt.float16
U64 = mybir.dt.uint64
I16 = mybir.dt.int16
OP = mybir.AluOpType
TWO23 = 12582912.0  # 1.5 * 2^23: forces round-to-integer in f32 for |x| < 2^22


def build_nc(fold1_split=None):
    """fold1_split: list of 'v'(vector) or 'g'(gpsimd) per chunk (len 8)."""
    if fold1_split is None:
        env = _os.environ.get("V2_SPLIT")
        if env:
            fold1_split = list(env)
        elif NCHUNK == 16:
            fold1_split = list("gvvgvvvgvvgvvvgv")
        else:
            fold1_split = ["v", "g"] * (NCHUNK // 2)
    assert len(fold1_split) == NCHUNK
    nc = bacc.Bacc("TRN2")
    # u32-declared (JAX canonicalizes u64 params); gathered as a u64 view
    patches = nc.declare_dram_parameter("patches", [PAIRS, NROWS, 64], mybir.dt.uint32, isOutput=False)
    offn = nc.declare_dram_parameter("offn", [PAIRS, 2, H, W], F32, isOutput=False)
    basen = nc.declare_dram_parameter("basen", [H, W], F32, isOutput=False)
    out = nc.declare_dram_parameter("out", [PAIRS, H, W, C], F16, isOutput=True)

    from contextlib import ExitStack

    with ExitStack() as stack:
        ec = stack.enter_context
        block = ec(nc.Block())
        Gb = [ec(nc.sbuf_tensor(f"G{i}", [128, WCH, 64], mybir.dt.uint32)) for i in range(NG)]
        Pb = [ec(nc.sbuf_tensor(f"P{i}", [128, WCH, C, 4], F16)) for i in range(NP_)]
        Ab = [ec(nc.sbuf_tensor(f"A{i}", [128, WCH, C, 2], F16)) for i in range(NA)]
        Bb = [ec(nc.sbuf_tensor(f"B{i}", [128, WCH, C], F16)) for i in range(NB)]
        onb = [ec(nc.sbuf_tensor(f"on{i}", [128, 2, W], F32)) for i in range(PAIRS)]
        bnat = ec(nc.sbuf_tensor("bnat", [128, W], F32))
        sy2b = [ec(nc.sbuf_tensor(f"sy2_{i}", [128, 2, W], F32)) for i in range(PAIRS)]
        sfb = [ec(nc.sbuf_tensor(f"sf{i}", [128, 2, W], F32)) for i in range(PAIRS)]
        sgb = [ec(nc.sbuf_tensor(f"sg{i}", [128, 2, W], F32)) for i in range(PAIRS)]
        wtkb = [ec(nc.sbuf_tensor(f"wtk{i}", [128, W, 4], F16)) for i in range(PAIRS)]
        tD = ec(nc.sbuf_tensor("tD", [128, W], F32))
        dnatb = [ec(nc.sbuf_tensor(f"dnat{i}", [128, W], I16)) for i in range(PAIRS)]
        d0 = ec(nc.sbuf_tensor("d0", [128, 1024], I16))
        d1 = ec(nc.sbuf_tensor("d1", [128, 1024], I16))
        db = [d0, d1]

        s_inb = ec(nc.semaphore("s_inb"))    # basen DMA
        s_inx = [ec(nc.semaphore(f"s_in{i}")) for i in range(PAIRS)]  # offn DMAs
        s_frac = ec(nc.semaphore("s_frac"))  # Pool fracs done (1/pair)
        s_dn = ec(nc.semaphore("s_dn"))      # dnat ready (1/pair)
        s_wr = ec(nc.semaphore("s_wr"))      # ACT wrap progress (68/pair)
        s_rp = ec(nc.semaphore("s_rp"))      # ACT replica DMAs (16/pair)
        s_g = [ec(nc.semaphore(f"s_g{i}")) for i in range(NCHUNK)]    # gather done (16)
        s_m = [ec(nc.semaphore(f"s_m{i}")) for i in range(NCHUNK)]    # mul done (1)
        s_f1 = [ec(nc.semaphore(f"s_f1_{i}")) for i in range(NCHUNK)]  # fold1 done (1)
        s_f2 = [ec(nc.semaphore(f"s_f2_{i}")) for i in range(NCHUNK)]  # fold2 done (1)
        s_out = [ec(nc.semaphore(f"s_out{i}")) for i in range(NCHUNK)]  # out DMA done (16)
        s_cv = ec(nc.semaphore("s_cv"))      # DVE same-engine chain
        s_cg = ec(nc.semaphore("s_cg"))      # Pool same-engine chain

        class Chain:
            """Serializes dependent ops on one engine via a chain semaphore."""

            def __init__(self, eng, sem):
                self.eng, self.sem, self.n = eng, sem, 0
                self.extra = []

            def run(self, thunk, final=None):
                if self.n:
                    self.eng.wait_ge(self.sem, self.n)
                for sem, val in self.extra:
                    self.eng.wait_ge(sem, val)
                self.extra = []
                inst = thunk()
                if final is None:
                    inst.then_inc(self.sem, 1)
                    self.n += 1
                else:
                    sem, val = final
                    inst.then_inc(sem, 1)
                    self.extra.append((sem, val))
                return inst

        @block.sync
        def _(sync: bass.BassEngine):
            sync.dma_start(onb[0][:, :, :], offn[0, :, :, :].transpose([1, 0, 2])).then_inc(s_inx[0], 16)
            sync.dma_start(bnat[:, :], basen[:, :]).then_inc(s_inb, 16)
            for p in range(1, PAIRS):
                sync.dma_start(onb[p][:, :, :], offn[p, :, :, :].transpose([1, 0, 2])).then_inc(s_inx[p], 16)
            for s in range(NCHUNK):
                p, c = divmod(s, CH)
                sync.wait_ge(s_f2[s], 1)
                dst = out[p, :, c * WCH:(c + 1) * WCH, :]   # (h, w, c)
                sync.dma_start(dst, Bb[s % NB][:, :, :]).then_inc(s_out[s], 16)

        def emit_idx_weights(ch, p):
            """DVE: gather indices dnat + fp16 weight products wtk for pair p."""
            eng = ch.eng
            sy2, sf, sg = sy2b[p], sfb[p], sgb[p]
            wtk = wtkb[p]
            dnat = dnatb[p]
            r = ch.run
            # idx first (unblocks ACT wrap asap)
            r(lambda: eng.scalar_tensor_tensor(tD[:, :], sy2[:, 0, :], float(HP), sy2[:, 1, :], OP.mult, OP.add))
            r(lambda: eng.tensor_add(dnat[:, :], tD[:, :], bnat[:, :]), final=(s_dn, p + 1))
            # weight products -> wtk[:, :, k], k order (00, 01, 10, 11)
            r(lambda: eng.tensor_mul(wtk[:, :, 0], sg[:, 0, :], sg[:, 1, :]))
            r(lambda: eng.tensor_mul(wtk[:, :, 1], sg[:, 0, :], sf[:, 1, :]))
            r(lambda: eng.tensor_mul(wtk[:, :, 2], sf[:, 0, :], sg[:, 1, :]))
            r(lambda: eng.tensor_mul(wtk[:, :, 3], sf[:, 0, :], sf[:, 1, :]))

        def emit_fold1(ch, s):
            P = Pb[s % NP_][:, :, :, :]
            A = Ab[s % NA]
            return ch.run(lambda: ch.eng.tensor_add(A[:, :, :, :], P[:, :, :, 0:2], P[:, :, :, 2:4]),
                          final=(s_f1[s], 1))

        @block.vector
        def _(vector: bass.BassEngine):
            ch = Chain(vector, s_cv)
            # zero the wrapped-idx tiles once
            ch.run(lambda: vector.memset(d0[:, :].bitcast(mybir.dt.uint32), 0))
            ch.run(lambda: vector.memset(d1[:, :].bitcast(mybir.dt.uint32), 0))
            vector.wait_ge(s_inb, 16)
            for p in range(PAIRS):
                vector.wait_ge(s_frac, p + 1)
                emit_idx_weights(ch, p)
            for s in range(NCHUNK):
                p, c = divmod(s, CH)
                # mul: P[s%2] = G4 * W4
                vector.wait_ge(s_g[s], 16)
                if s >= NP_ and fold1_split[s - NP_] == "g":
                    vector.wait_ge(s_f1[s - NP_], 1)   # P[s%NP_] free
                G4 = Gb[s % NG][:, :, :].bitcast(F16).rearrange("p w (c k) -> p w c k", k=4)
                W4 = wtkb[p][:, c * WCH:(c + 1) * WCH, None, :].broadcast_to([128, WCH, C, 4])
                P = Pb[s % NP_]
                ch.run(lambda G4=G4, W4=W4, P=P: vector.tensor_mul(P[:, :, :, :], G4, W4),
                       final=(s_m[s], 1))
                if fold1_split[s] == "v":
                    if s >= NA:
                        vector.wait_ge(s_f2[s - NA], 1)   # A[s%NA] free
                    emit_fold1(ch, s)

        @block.scalar
        def _(act: bass.BassEngine):
            # rearrange dnat [128h, 128w] -> wrapped d[st] partitions 0-31:
            # d[q, w*8+k] = dnat[q+16k, w]
            for p in range(PAIRS):
                st = p % 2
                if p >= 1:
                    act.wait_ge(s_wr, 84 * p)   # drain own prior-pair DMA incs
                act.wait_ge(s_dn, p + 1)
                if p >= 2:
                    # d[st] reuse: ALL gathers of pair p-2 must be done
                    for cc in range(CH):
                        act.wait_ge(s_g[CH * (p - 2) + cc], 16)
                dnat = dnatb[p]
                dwrap = db[st][:, :].rearrange("p (w k) -> p w k", k=8)
                for k in range(0, 8, 2):   # even k: engine copy (32-aligned src)
                    act.copy(dwrap[0:16, :, k],
                             dnat[16 * k:16 * (k + 1), :]).then_inc(s_wr, 1)
                with nc.allow_non_contiguous_dma(reason="4KB idx-wrap strided dst"):
                    for k in range(1, 8, 2):   # odd k: tiny DMA (no partition align)
                        act.dma_start(dwrap[0:16, :, k],
                                      dnat[16 * k:16 * (k + 1), :]).then_inc(s_wr, 16)
                act.wait_ge(s_wr, 84 * p + 68)
                # replicate wrapped indices to partitions 16-31 (HW Q7 cores
                # read that group; the simulator reads 0-15)
                act.dma_start(db[st][16:32, :], db[st][0:16, :]).then_inc(s_wr, 16)

        @block.gpsimd
        def _(gpsimd: bass.BassGpSimd):
            chg = Chain(gpsimd, s_cg)
            gpsimd.load_library(mlp)

            def frac_final_fix(p):
                # emit_fracs used final=(s_frac, 0) marker; replace with actual
                pass

            def emit_fracs_pool(p):
                gpsimd.wait_ge(s_inx[p], 16)
                onf = onb[p][:, :, :]
                sy2, sf, sg = sy2b[p], sfb[p], sgb[p]
                r = chg.run
                # floor(x) = round_ne(x-0.5) via the fp32 TWO23 trick.
                # Ties (x within ~1ulp of an integer) may floor one down, but
                # bilinear interpolation is continuous there: the weight
                # compensates the index exactly, so the output is unchanged.
                r(lambda: gpsimd.tensor_scalar(sy2[:, :, :], onf, -0.5, TWO23, OP.add, OP.add))
                r(lambda: gpsimd.tensor_scalar(sy2[:, :, :], sy2[:, :, :], -TWO23, 0.0, OP.add, OP.add))
                r(lambda: gpsimd.tensor_sub(sf[:, :, :], onf, sy2[:, :, :]))
                r(lambda: gpsimd.tensor_scalar(sg[:, :, :], sf[:, :, :], -1.0, 1.0, OP.mult, OP.add),
                  final=(s_frac, p + 1))

            def emit_gather(s):
                p, c = divmod(s, CH)
                gpsimd.wait_ge(s_wr, 84 * (p + 1))
                if s >= NG:
                    gpsimd.wait_ge(s_m[s - NG], 1)   # G[s%NG] free
                ic = NIDX_CH // 16
                gpsimd.dma_gather(
                    Gb[s % NG][:, :, :],
                    patches[p, :, :],
                    db[p % 2][:, c * ic:(c + 1) * ic],
                    NIDX_CH,
                    NIDX_CH,
                    64,
                    single_packet=False,
                ).then_inc(s_g[s], 16)

            def emit_fold2(s):
                A = Ab[s % NA]
                B = Bb[s % NB]
                gpsimd.wait_ge(s_f1[s], 1)
                if s >= NB:
                    gpsimd.wait_ge(s_out[s - NB], 16)   # B[s%NB] free
                chg.run(lambda: gpsimd.tensor_add(B[:, :, :], A[:, :, :, 0], A[:, :, :, 1]),
                        final=(s_f2[s], 1))

            def emit_fold1_pool(s):
                gpsimd.wait_ge(s_m[s], 1)
                if s >= NA:
                    gpsimd.wait_ge(s_f2[s - NA], 1)   # A[s%NA] free
                emit_fold1(chg, s)

            # static schedule: fracs interleaved with gathers and folds
            order_env = _os.environ.get("V2_POOL_ORDER")
            lag = int(_os.environ.get("V2_LAG", "2"))
            if order_env:
                order = [tuple(tok.split(":")) for tok in order_env.split(",")]
                order = [(a, int(b)) for a, b in order]
            else:
                order = [("fr", p) for p in range(PAIRS)]
                done = 0
                for s in range(NCHUNK):
                    order.append(("g", s))
                    while done <= s - lag:
                        order.append(("f1", done))
                        order.append(("f2", done))
                        done += 1
                while done < NCHUNK:
                    order.append(("f1", done))
                    order.append(("f2", done))
                    done += 1
            for kind, i in order:
                if kind == "fr":
                    emit_fracs_pool(i)
                elif kind == "g":
                    emit_gather(i)
                elif kind == "f1":
                    if fold1_split[i] == "g":
                        emit_fold1_pool(i)
                elif kind == "f2":
                    emit_fold2(i)

    nc.compile()
    return nc


# ---------------- host-side helpers ----------------

def build_patches_all(imgs_pairs):
    """imgs_pairs: (NPAIR, C, H, W) f32 -> (NPAIR, NROWS, 32) u64.

    Row at anchor (hh, ww) = fp16[c][k]: c-major, 4 corners packed per
    channel: k order (0,0), (0,1), (1,0), (1,1)."""
    npair = imgs_pairs.shape[0]
    hw_c = np.ascontiguousarray(np.transpose(imgs_pairs, (0, 2, 3, 1))).astype(np.float16)
    padded = np.zeros((npair, HP + 1, HP + 1, C), np.float16)
    padded[:, PAD:PAD + H, PAD:PAD + W] = hw_c
    P = np.empty((npair, HP, HP, C, 4), np.float16)
    P[:, :, :, :, 0] = padded[:, 0:HP, 0:HP]
    P[:, :, :, :, 1] = padded[:, 0:HP, 1:HP + 1]
    P[:, :, :, :, 2] = padded[:, 1:HP + 1, 0:HP]
    P[:, :, :, :, 3] = padded[:, 1:HP + 1, 1:HP + 1]
    return np.ascontiguousarray(P).reshape(npair, NROWS, 128).view(np.uint32)


def base_natural():
    h = np.arange(H).reshape(H, 1)
    w = np.arange(W).reshape(1, W)
    return ((h + PAD) * HP + (w + PAD)).astype(np.float32)


def make_in_map(imgs_pairs, offp):
    return {
        "patches": build_patches_all(imgs_pairs),
        "offn": np.ascontiguousarray(offp),
        "basen": base_natural(),
    }


# ---------------- public entry point ----------------

N_CORES = 8
PAIRS_TOTAL = 32

LAST_EXEC_TIME_NS = None


def kernel(images, offsets):
    """images (4,8,32,128,128) f32; offsets (4,16,128,128) f32 ->
    (4,8,32,128,128) f32 deformable bilinear sampling, on 8 NeuronCores."""
    import os
    global LAST_EXEC_TIME_NS
    from concourse.bass_utils import run_bass_kernel_spmd

    images = np.ascontiguousarray(np.asarray(images, dtype=np.float32))
    offsets = np.ascontiguousarray(np.asarray(offsets, dtype=np.float32))
    imgs = images.reshape(PAIRS_TOTAL, C, H, W)
    offp = offsets.reshape(4, 8, 2, H, W).reshape(PAIRS_TOTAL, 2, H, W)

    nc = build_nc()
    in_maps = []
    for core in range(N_CORES):
        sl = slice(core * PAIRS, (core + 1) * PAIRS)
        in_maps.append(make_in_map(imgs[sl], offp[sl]))
    trace = bool(os.environ.get("DK_TRACE"))
    res = run_bass_kernel_spmd(nc, in_maps, list(range(N_CORES)), trace=trace)
    if trace:
        LAST_EXEC_TIME_NS = res.exec_time_ns
        if res.instructions_and_trace:
            print("trace path:", res.instructions_and_trace[1])
    outs = [np.asarray(res.results[i]["out"]) for i in range(N_CORES)]
    full = np.concatenate(outs, axis=0).astype(np.float32)   # (32, H, W, C)
    full = np.transpose(full, (0, 3, 1, 2))                  # (32, C, H, W)
    return np.ascontiguousarray(full.reshape(4, 8, C, H, W)).astype(np.float32)


# revision 9
# speedup vs baseline: 2.1215x; 1.0112x over previous
"""Deformable bilinear sampling kernel for TRN2 (8-core SPMD), v2.

Per (n,o) pair, each output pixel (h,w) needs the 2x2xC patch at
(h+floor(off_h), w+floor(off_w)) with bilinear corner weights. The host stages
a patch tensor P[pair] where row (hh*144+ww) holds the 256B fp16 patch at
padded anchor (hh,ww), laid out c-major with the 4 corners packed per channel
(so one u64 = one channel's 4 corners). The device computes int16 gather
indices on DVE + corner-weight products, pulls one 256B row per pixel with
gpsimd.dma_gather (u32-aliased: 64 "elements"/row), then combines with a
single 2x-mode fused multiply (k packed last) + two tree adds, and writes
fp16 output.

Engine split: Pool = fracs math, gathers, fold2 (+some fold1); DVE = idx math,
weight products, muls, fold1; ACT = idx wrap; SP = all input/output DMAs.
"""

import numpy as np

import concourse.bacc as bacc
import concourse.bass as bass
import concourse.mybir as mybir
from concourse.library_config import mlp

import os as _os

PAIRS = 4          # (n,o) pairs per core
H = W = 128
C = 32
PAD = 8
HP = 144           # padded anchor grid
NROWS = HP * HP    # 20736 patch rows per pair
NIDX = H * W       # 16384 gathered pixels per pair
CH = int(_os.environ.get("V2_CH", "4"))   # gather chunks per pair
NIDX_CH = NIDX // CH
WCH = W // CH      # w-columns per chunk
NCHUNK = PAIRS * CH
NG = int(_os.environ.get("V2_NG", "6"))   # gather buffers
NP_ = int(_os.environ.get("V2_NP", "3"))  # product buffers
NA = int(_os.environ.get("V2_NA", "3"))   # fold1 buffers
NB = int(_os.environ.get("V2_NB", "6"))   # out buffers

F32 = mybir.dt.float32
F16 = mybir.dt.float16
U64 = mybir.dt.uint64
I16 = mybir.dt.int16
OP = mybir.AluOpType
TWO23 = 12582912.0  # 1.5 * 2^23: forces round-to-integer in f32 for |x| < 2^22


def build_nc(fold1_split=None):
    """fold1_split: list of 'v'(vector) or 'g'(gpsimd) per chunk (len 8)."""
    if fold1_split is None:
        env = _os.environ.get("V2_SPLIT")
        if env:
            fold1_split = list(env)
        elif NCHUNK == 16:
            fold1_split = list("gvvgvvvgvvgvvvgv")
        else:
            fold1_split = ["v", "g"] * (NCHUNK // 2)
    assert len(fold1_split) == NCHUNK
    nc = bacc.Bacc("TRN2")
    # u32-declared (JAX canonicalizes u64 params); gathered as a u64 view
    patches = nc.declare_dram_parameter("patches", [PAIRS, NROWS, 64], mybir.dt.uint32, isOutput=False)
    offn = nc.declare_dram_parameter("offn", [PAIRS, 2, H, W], F32, isOutput=False)
    basen = nc.declare_dram_parameter("basen", [H, W], F32, isOutput=False)
    out = nc.declare_dram_parameter("out", [PAIRS, H, W, C], F16, isOutput=True)

    from contextlib import ExitStack

    with ExitStack() as stack:
        ec = stack.enter_context
        block = ec(nc.Block())
        Gb = [ec(nc.sbuf_tensor(f"G{i}", [128, WCH, 64], mybir.dt.uint32)) for i in range(NG)]
        Pb = [ec(nc.sbuf_tensor(f"P{i}", [128, WCH, C, 4], F16)) for i in range(NP_)]
        Ab = [ec(nc.sbuf_tensor(f"A{i}", [128, WCH, C, 2], F16)) for i in range(NA)]
        Bb = [ec(nc.sbuf_tensor(f"B{i}", [128, WCH, C], F16)) for i in range(NB)]
        onb = [ec(nc.sbuf_tensor(f"on{i}", [128, 2, W], F32)) for i in range(PAIRS)]
        bnat = ec(nc.sbuf_tensor("bnat", [128, W], F32))
        sy2b = [ec(nc.sbuf_tensor(f"sy2_{i}", [128, 2, W], F32)) for i in range(PAIRS)]
        sfb = [ec(nc.sbuf_tensor(f"sf{i}", [128, 2, W], F32)) for i in range(PAIRS)]
        sgb = [ec(nc.sbuf_tensor(f"sg{i}", [128, 2, W], F32)) for i in range(PAIRS)]
        wtkb = [ec(nc.sbuf_tensor(f"wtk{i}", [128, W, 4], F16)) for i in range(PAIRS)]
        tD = ec(nc.sbuf_tensor("tD", [128, W], F32))
        dnatb = [ec(nc.sbuf_tensor(f"dnat{i}", [128, W], I16)) for i in range(PAIRS)]
        d0 = ec(nc.sbuf_tensor("d0", [128, 1024], I16))
        d1 = ec(nc.sbuf_tensor("d1", [128, 1024], I16))
        db = [d0, d1]

        s_inb = ec(nc.semaphore("s_inb"))    # basen DMA
        s_inx = [ec(nc.semaphore(f"s_in{i}")) for i in range(PAIRS)]  # offn DMAs
        s_frac = ec(nc.semaphore("s_frac"))  # Pool fracs done (1/pair)
        s_dn = ec(nc.semaphore("s_dn"))      # dnat ready (1/pair)
        s_wr = ec(nc.semaphore("s_wr"))      # ACT wrap: pair-0 first w-half (84)
        s_wr2 = ec(nc.semaphore("s_wr2"))    # ACT wrap: rest (84 per unit)
        s_g = [ec(nc.semaphore(f"s_g{i}")) for i in range(NCHUNK)]    # gather done (16)
        s_m = [ec(nc.semaphore(f"s_m{i}")) for i in range(NCHUNK)]    # mul done (1)
        s_f1 = [ec(nc.semaphore(f"s_f1_{i}")) for i in range(NCHUNK)]  # fold1 done (1)
        s_f2 = [ec(nc.semaphore(f"s_f2_{i}")) for i in range(NCHUNK)]  # fold2 done (1)
        s_out = [ec(nc.semaphore(f"s_out{i}")) for i in range(NCHUNK)]  # out DMA done (16)
        s_cv = ec(nc.semaphore("s_cv"))      # DVE same-engine chain
        s_cg = ec(nc.semaphore("s_cg"))      # Pool same-engine chain

        class Chain:
            """Serializes dependent ops on one engine via a chain semaphore."""

            def __init__(self, eng, sem):
                self.eng, self.sem, self.n = eng, sem, 0
                self.extra = []

            def run(self, thunk, final=None):
                if self.n:
                    self.eng.wait_ge(self.sem, self.n)
                for sem, val in self.extra:
                    self.eng.wait_ge(sem, val)
                self.extra = []
                inst = thunk()
                if final is None:
                    inst.then_inc(self.sem, 1)
                    self.n += 1
                else:
                    sem, val = final
                    inst.then_inc(sem, 1)
                    self.extra.append((sem, val))
                return inst

        @block.sync
        def _(sync: bass.BassEngine):
            sync.dma_start(onb[0][:, :, :], offn[0, :, :, :].transpose([1, 0, 2])).then_inc(s_inx[0], 16)
            sync.dma_start(bnat[:, :], basen[:, :]).then_inc(s_inb, 16)
            for p in range(1, PAIRS):
                sync.dma_start(onb[p][:, :, :], offn[p, :, :, :].transpose([1, 0, 2])).then_inc(s_inx[p], 16)
            for s in range(NCHUNK):
                p, c = divmod(s, CH)
                sync.wait_ge(s_f2[s], 1)
                dst = out[p, :, c * WCH:(c + 1) * WCH, :]   # (h, w, c)
                sync.dma_start(dst, Bb[s % NB][:, :, :]).then_inc(s_out[s], 16)

        def emit_idx_weights(ch, p):
            """DVE: gather indices dnat + fp16 weight products wtk for pair p."""
            eng = ch.eng
            sy2, sf, sg = sy2b[p], sfb[p], sgb[p]
            wtk = wtkb[p]
            dnat = dnatb[p]
            r = ch.run
            # idx first (unblocks ACT wrap asap)
            r(lambda: eng.scalar_tensor_tensor(tD[:, :], sy2[:, 0, :], float(HP), sy2[:, 1, :], OP.mult, OP.add))
            r(lambda: eng.tensor_add(dnat[:, :], tD[:, :], bnat[:, :]), final=(s_dn, p + 1))
            # weight products -> wtk[:, :, k], k order (00, 01, 10, 11)
            r(lambda: eng.tensor_mul(wtk[:, :, 0], sg[:, 0, :], sg[:, 1, :]))
            r(lambda: eng.tensor_mul(wtk[:, :, 1], sg[:, 0, :], sf[:, 1, :]))
            r(lambda: eng.tensor_mul(wtk[:, :, 2], sf[:, 0, :], sg[:, 1, :]))
            r(lambda: eng.tensor_mul(wtk[:, :, 3], sf[:, 0, :], sf[:, 1, :]))

        def emit_fold1(ch, s):
            P = Pb[s % NP_][:, :, :, :]
            A = Ab[s % NA]
            return ch.run(lambda: ch.eng.tensor_add(A[:, :, :, :], P[:, :, :, 0:2], P[:, :, :, 2:4]),
                          final=(s_f1[s], 1))

        @block.vector
        def _(vector: bass.BassEngine):
            ch = Chain(vector, s_cv)
            # zero the wrapped-idx tiles once
            ch.run(lambda: vector.memset(d0[:, :].bitcast(mybir.dt.uint32), 0))
            ch.run(lambda: vector.memset(d1[:, :].bitcast(mybir.dt.uint32), 0))
            vector.wait_ge(s_inb, 16)
            for p in range(PAIRS):
                vector.wait_ge(s_frac, p + 1)
                emit_idx_weights(ch, p)
            for s in range(NCHUNK):
                p, c = divmod(s, CH)
                # mul: P[s%2] = G4 * W4
                vector.wait_ge(s_g[s], 16)
                if s >= NP_ and fold1_split[s - NP_] == "g":
                    vector.wait_ge(s_f1[s - NP_], 1)   # P[s%NP_] free
                G4 = Gb[s % NG][:, :, :].bitcast(F16).rearrange("p w (c k) -> p w c k", k=4)
                W4 = wtkb[p][:, c * WCH:(c + 1) * WCH, None, :].broadcast_to([128, WCH, C, 4])
                P = Pb[s % NP_]
                ch.run(lambda G4=G4, W4=W4, P=P: vector.tensor_mul(P[:, :, :, :], G4, W4),
                       final=(s_m[s], 1))
                if fold1_split[s] == "v":
                    if s >= NA:
                        vector.wait_ge(s_f2[s - NA], 1)   # A[s%NA] free
                    emit_fold1(ch, s)

        @block.scalar
        def _(act: bass.BassEngine):
            # rearrange dnat [128h, 128w] -> wrapped d[st] partitions 0-31:
            # d[q, w*8+k] = dnat[q+16k, w]. Pair 0 wraps in two w-halves so
            # the first gathers can start after half 0 (sem s_wr); everything
            # later counts on s_wr2 in 84-inc units.
            def emit_wrap(p, w0, w1, sem):
                st = p % 2
                dnat = dnatb[p]
                dwrap = db[st][:, :].rearrange("p (w k) -> p w k", k=8)
                for k in range(0, 8, 2):   # even k: engine copy (32-aligned src)
                    act.copy(dwrap[0:16, w0:w1, k],
                             dnat[16 * k:16 * (k + 1), w0:w1]).then_inc(sem, 1)
                with nc.allow_non_contiguous_dma(reason="4KB idx-wrap strided dst"):
                    for k in range(1, 8, 2):   # odd k: tiny DMA (no partition align)
                        act.dma_start(dwrap[0:16, w0:w1, k],
                                      dnat[16 * k:16 * (k + 1), w0:w1]).then_inc(sem, 16)

            def emit_repl(p, w0, w1, sem, seen):
                st = p % 2
                act.wait_ge(sem, seen)
                act.dma_start(db[st][16:32, w0 * 8:w1 * 8],
                              db[st][0:16, w0 * 8:w1 * 8]).then_inc(sem, 16)

            act.wait_ge(s_dn, 1)
            emit_wrap(0, 0, 64, s_wr)
            emit_repl(0, 0, 64, s_wr, 68)
            emit_wrap(0, 64, 128, s_wr2)
            emit_repl(0, 64, 128, s_wr2, 68)
            for p in range(1, PAIRS):
                act.wait_ge(s_wr2, 84 * p)   # drain own prior DMA incs
                act.wait_ge(s_dn, p + 1)
                if p >= 2:
                    # d[st] reuse: ALL gathers of pair p-2 must be done
                    for cc in range(CH):
                        act.wait_ge(s_g[CH * (p - 2) + cc], 16)
                emit_wrap(p, 0, 128, s_wr2)
                emit_repl(p, 0, 128, s_wr2, 84 * p + 68)

        @block.gpsimd
        def _(gpsimd: bass.BassGpSimd):
            chg = Chain(gpsimd, s_cg)
            gpsimd.load_library(mlp)

            def frac_final_fix(p):
                # emit_fracs used final=(s_frac, 0) marker; replace with actual
                pass

            def emit_fracs_pool(p):
                gpsimd.wait_ge(s_inx[p], 16)
                onf = onb[p][:, :, :]
                sy2, sf, sg = sy2b[p], sfb[p], sgb[p]
                r = chg.run
                # floor(x) = round_ne(x-0.5) via the fp32 TWO23 trick.
                # Ties (x within ~1ulp of an integer) may floor one down, but
                # bilinear interpolation is continuous there: the weight
                # compensates the index exactly, so the output is unchanged.
                r(lambda: gpsimd.tensor_scalar(sy2[:, :, :], onf, -0.5, TWO23, OP.add, OP.add))
                r(lambda: gpsimd.tensor_scalar(sy2[:, :, :], sy2[:, :, :], -TWO23, 0.0, OP.add, OP.add))
                r(lambda: gpsimd.tensor_sub(sf[:, :, :], onf, sy2[:, :, :]))
                r(lambda: gpsimd.tensor_scalar(sg[:, :, :], sf[:, :, :], -1.0, 1.0, OP.mult, OP.add),
                  final=(s_frac, p + 1))

            def emit_gather(s):
                p, c = divmod(s, CH)
                if p == 0 and c < CH // 2:
                    gpsimd.wait_ge(s_wr, 84)       # pair-0 first w-half wrapped
                else:
                    gpsimd.wait_ge(s_wr2, 84 * (p + 1))
                if s >= NG:
                    gpsimd.wait_ge(s_m[s - NG], 1)   # G[s%NG] free
                ic = NIDX_CH // 16
                gpsimd.dma_gather(
                    Gb[s % NG][:, :, :],
                    patches[p, :, :],
                    db[p % 2][:, c * ic:(c + 1) * ic],
                    NIDX_CH,
                    NIDX_CH,
                    64,
                    single_packet=False,
                ).then_inc(s_g[s], 16)

            def emit_fold2(s):
                A = Ab[s % NA]
                B = Bb[s % NB]
                gpsimd.wait_ge(s_f1[s], 1)
                if s >= NB:
                    gpsimd.wait_ge(s_out[s - NB], 16)   # B[s%NB] free
                chg.run(lambda: gpsimd.tensor_add(B[:, :, :], A[:, :, :, 0], A[:, :, :, 1]),
                        final=(s_f2[s], 1))

            def emit_fold1_pool(s):
                gpsimd.wait_ge(s_m[s], 1)
                if s >= NA:
                    gpsimd.wait_ge(s_f2[s - NA], 1)   # A[s%NA] free
                emit_fold1(chg, s)

            # static schedule: fracs interleaved with gathers and folds
            order_env = _os.environ.get("V2_POOL_ORDER")
            lag = int(_os.environ.get("V2_LAG", "2"))
            if order_env:
                order = [tuple(tok.split(":")) for tok in order_env.split(",")]
                order = [(a, int(b)) for a, b in order]
            else:
                order = [("fr", p) for p in range(PAIRS)]
                done = 0
                for s in range(NCHUNK):
                    order.append(("g", s))
                    while done <= s - lag:
                        order.append(("f1", done))
                        order.append(("f2", done))
                        done += 1
                while done < NCHUNK:
                    order.append(("f1", done))
                    order.append(("f2", done))
                    done += 1
            for kind, i in order:
                if kind == "fr":
                    emit_fracs_pool(i)
                elif kind == "g":
                    emit_gather(i)
                elif kind == "f1":
                    if fold1_split[i] == "g":
                        emit_fold1_pool(i)
                elif kind == "f2":
                    emit_fold2(i)

    nc.compile()
    return nc


# ---------------- host-side helpers ----------------

def build_patches_all(imgs_pairs):
    """imgs_pairs: (NPAIR, C, H, W) f32 -> (NPAIR, NROWS, 32) u64.

    Row at anchor (hh, ww) = fp16[c][k]: c-major, 4 corners packed per
    channel: k order (0,0), (0,1), (1,0), (1,1)."""
    npair = imgs_pairs.shape[0]
    hw_c = np.ascontiguousarray(np.transpose(imgs_pairs, (0, 2, 3, 1))).astype(np.float16)
    padded = np.zeros((npair, HP + 1, HP + 1, C), np.float16)
    padded[:, PAD:PAD + H, PAD:PAD + W] = hw_c
    P = np.empty((npair, HP, HP, C, 4), np.float16)
    P[:, :, :, :, 0] = padded[:, 0:HP, 0:HP]
    P[:, :, :, :, 1] = padded[:, 0:HP, 1:HP + 1]
    P[:, :, :, :, 2] = padded[:, 1:HP + 1, 0:HP]
    P[:, :, :, :, 3] = padded[:, 1:HP + 1, 1:HP + 1]
    return np.ascontiguousarray(P).reshape(npair, NROWS, 128).view(np.uint32)


def base_natural():
    h = np.arange(H).reshape(H, 1)
    w = np.arange(W).reshape(1, W)
    return ((h + PAD) * HP + (w + PAD)).astype(np.float32)


def make_in_map(imgs_pairs, offp):
    return {
        "patches": build_patches_all(imgs_pairs),
        "offn": np.ascontiguousarray(offp),
        "basen": base_natural(),
    }


# ---------------- public entry point ----------------

N_CORES = 8
PAIRS_TOTAL = 32

LAST_EXEC_TIME_NS = None


def kernel(images, offsets):
    """images (4,8,32,128,128) f32; offsets (4,16,128,128) f32 ->
    (4,8,32,128,128) f32 deformable bilinear sampling, on 8 NeuronCores."""
    import os
    global LAST_EXEC_TIME_NS
    from concourse.bass_utils import run_bass_kernel_spmd

    images = np.ascontiguousarray(np.asarray(images, dtype=np.float32))
    offsets = np.ascontiguousarray(np.asarray(offsets, dtype=np.float32))
    imgs = images.reshape(PAIRS_TOTAL, C, H, W)
    offp = offsets.reshape(4, 8, 2, H, W).reshape(PAIRS_TOTAL, 2, H, W)

    nc = build_nc()
    in_maps = []
    for core in range(N_CORES):
        sl = slice(core * PAIRS, (core + 1) * PAIRS)
        in_maps.append(make_in_map(imgs[sl], offp[sl]))
    trace = bool(os.environ.get("DK_TRACE"))
    res = run_bass_kernel_spmd(nc, in_maps, list(range(N_CORES)), trace=trace)
    if trace:
        LAST_EXEC_TIME_NS = res.exec_time_ns
        if res.instructions_and_trace:
            print("trace path:", res.instructions_and_trace[1])
    outs = [np.asarray(res.results[i]["out"]) for i in range(N_CORES)]
    full = np.concatenate(outs, axis=0).astype(np.float32)   # (32, H, W, C)
    full = np.transpose(full, (0, 3, 1, 2))                  # (32, C, H, W)
    return np.ascontiguousarray(full.reshape(4, 8, C, H, W)).astype(np.float32)


# revision 10
# speedup vs baseline: 2.2004x; 1.0372x over previous
"""Deformable bilinear sampling kernel for TRN2 (8-core SPMD), v2.

Per (n,o) pair, each output pixel (h,w) needs the 2x2xC patch at
(h+floor(off_h), w+floor(off_w)) with bilinear corner weights. The host stages
a patch tensor P[pair] where row (hh*144+ww) holds the 256B fp16 patch at
padded anchor (hh,ww), laid out c-major with the 4 corners packed per channel
(so one u64 = one channel's 4 corners). The device computes int16 gather
indices on DVE + corner-weight products, pulls one 256B row per pixel with
gpsimd.dma_gather (u32-aliased: 64 "elements"/row), then combines with a
single 2x-mode fused multiply (k packed last) + two tree adds, and writes
fp16 output.

Engine split: Pool = fracs math, gathers, fold2 (+some fold1); DVE = idx math,
weight products, muls, fold1; ACT = idx wrap; SP = all input/output DMAs.
"""

import numpy as np

import concourse.bacc as bacc
import concourse.bass as bass
import concourse.mybir as mybir
from concourse.library_config import mlp

import os as _os

PAIRS = 4          # (n,o) pairs per core
H = W = 128
C = 32
PAD = 8
HP = 144           # padded anchor grid
NROWS = HP * HP    # 20736 patch rows per pair
NIDX = H * W       # 16384 gathered pixels per pair
CH = int(_os.environ.get("V2_CH", "4"))   # gather chunks per pair
NIDX_CH = NIDX // CH
WCH = W // CH      # w-columns per chunk
NCHUNK = PAIRS * CH
NG = int(_os.environ.get("V2_NG", "6"))   # gather buffers
NP_ = int(_os.environ.get("V2_NP", "3"))  # product buffers
NA = int(_os.environ.get("V2_NA", "3"))   # fold1 buffers
NB = int(_os.environ.get("V2_NB", "6"))   # out buffers

F32 = mybir.dt.float32
F16 = mybir.dt.float16
U64 = mybir.dt.uint64
I16 = mybir.dt.int16
OP = mybir.AluOpType
TWO23 = 12582912.0  # 1.5 * 2^23: forces round-to-integer in f32 for |x| < 2^22


def build_nc(fold1_split=None):
    """fold1_split: list of 'v'(vector) or 'g'(gpsimd) per chunk (len 8)."""
    if fold1_split is None:
        env = _os.environ.get("V2_SPLIT")
        if env:
            fold1_split = list(env)
        elif NCHUNK == 16:
            fold1_split = list("gvvgvvvgvvgvvvgv")
        else:
            fold1_split = ["v", "g"] * (NCHUNK // 2)
    assert len(fold1_split) == NCHUNK
    nc = bacc.Bacc("TRN2")
    # u32-declared (JAX canonicalizes u64 params); gathered as a u64 view
    patches = nc.declare_dram_parameter("patches", [PAIRS, NROWS, 64], mybir.dt.uint32, isOutput=False)
    offn = nc.declare_dram_parameter("offn", [PAIRS, 2, H, W], F32, isOutput=False)
    basen = nc.declare_dram_parameter("basen", [H, W], F32, isOutput=False)
    out = nc.declare_dram_parameter("out", [PAIRS, H, W, C], F16, isOutput=True)

    from contextlib import ExitStack

    with ExitStack() as stack:
        ec = stack.enter_context
        block = ec(nc.Block())
        Gb = [ec(nc.sbuf_tensor(f"G{i}", [128, WCH, 64], mybir.dt.uint32)) for i in range(NG)]
        Pb = [ec(nc.sbuf_tensor(f"P{i}", [128, WCH, C, 4], F16)) for i in range(NP_)]
        Ab = [ec(nc.sbuf_tensor(f"A{i}", [128, WCH, C, 2], F16)) for i in range(NA)]
        Bb = [ec(nc.sbuf_tensor(f"B{i}", [128, WCH, C], F16)) for i in range(NB)]
        onb = [ec(nc.sbuf_tensor(f"on{i}", [128, 2, W], F32)) for i in range(PAIRS)]
        bnat = ec(nc.sbuf_tensor("bnat", [128, W], F32))
        sy2b = [ec(nc.sbuf_tensor(f"sy2_{i}", [128, 2, W], F32)) for i in range(PAIRS)]
        sfb = [ec(nc.sbuf_tensor(f"sf{i}", [128, 2, W], F32)) for i in range(PAIRS)]
        sgb = [ec(nc.sbuf_tensor(f"sg{i}", [128, 2, W], F32)) for i in range(PAIRS)]
        wtkb = [ec(nc.sbuf_tensor(f"wtk{i}", [128, W, 4], F16)) for i in range(PAIRS)]
        tD = ec(nc.sbuf_tensor("tD", [128, W], F32))
        dnatb = [ec(nc.sbuf_tensor(f"dnat{i}", [128, W], I16)) for i in range(PAIRS)]
        d0 = ec(nc.sbuf_tensor("d0", [128, 1024], I16))
        d1 = ec(nc.sbuf_tensor("d1", [128, 1024], I16))
        db = [d0, d1]

        s_inb = ec(nc.semaphore("s_inb"))    # basen DMA
        s_inx = [ec(nc.semaphore(f"s_in{i}")) for i in range(PAIRS)]  # offn DMAs
        s_frac = ec(nc.semaphore("s_frac"))  # Pool fracs done (1/pair)
        s_dn = ec(nc.semaphore("s_dn"))      # dnat ready (1/pair)
        s_wr = ec(nc.semaphore("s_wr"))      # ACT wrap: pair-0 first w-half (84)
        s_wr2 = ec(nc.semaphore("s_wr2"))    # ACT wrap: rest (84 per unit)
        s_g = [ec(nc.semaphore(f"s_g{i}")) for i in range(NCHUNK)]    # gather done (16)
        s_m = [ec(nc.semaphore(f"s_m{i}")) for i in range(NCHUNK)]    # mul done (1)
        s_f1 = [ec(nc.semaphore(f"s_f1_{i}")) for i in range(NCHUNK)]  # fold1 done (1)
        s_f2 = [ec(nc.semaphore(f"s_f2_{i}")) for i in range(NCHUNK)]  # fold2 done (1)
        s_out = [ec(nc.semaphore(f"s_out{i}")) for i in range(NCHUNK)]  # out DMA done (16)
        s_cv = ec(nc.semaphore("s_cv"))      # DVE same-engine chain
        s_cg = ec(nc.semaphore("s_cg"))      # Pool same-engine chain

        class Chain:
            """Serializes dependent ops on one engine via a chain semaphore."""

            def __init__(self, eng, sem):
                self.eng, self.sem, self.n = eng, sem, 0
                self.extra = []

            def run(self, thunk, final=None):
                if self.n:
                    self.eng.wait_ge(self.sem, self.n)
                for sem, val in self.extra:
                    self.eng.wait_ge(sem, val)
                self.extra = []
                inst = thunk()
                if final is None:
                    inst.then_inc(self.sem, 1)
                    self.n += 1
                else:
                    sem, val = final
                    inst.then_inc(sem, 1)
                    self.extra.append((sem, val))
                return inst

        @block.sync
        def _(sync: bass.BassEngine):
            sync.dma_start(onb[0][:, :, :], offn[0, :, :, :].transpose([1, 0, 2])).then_inc(s_inx[0], 16)
            sync.dma_start(bnat[:, :], basen[:, :]).then_inc(s_inb, 16)
            for p in range(1, PAIRS):
                sync.dma_start(onb[p][:, :, :], offn[p, :, :, :].transpose([1, 0, 2])).then_inc(s_inx[p], 16)
            for s in range(NCHUNK):
                p, c = divmod(s, CH)
                if s == NCHUNK - 1:
                    h = WCH // 2
                    sync.wait_ge(s_f2[s], 1)
                    sync.dma_start(out[p, :, c * WCH:c * WCH + h, :],
                                   Bb[s % NB][:, 0:h, :]).then_inc(s_out[s], 16)
                    sync.wait_ge(s_f2[s], 2)
                    sync.dma_start(out[p, :, c * WCH + h:(c + 1) * WCH, :],
                                   Bb[s % NB][:, h:WCH, :]).then_inc(s_out[s], 16)
                else:
                    sync.wait_ge(s_f2[s], 1)
                    dst = out[p, :, c * WCH:(c + 1) * WCH, :]   # (h, w, c)
                    sync.dma_start(dst, Bb[s % NB][:, :, :]).then_inc(s_out[s], 16)

        def emit_idx_weights(ch, p):
            """DVE: gather indices dnat + fp16 weight products wtk for pair p."""
            eng = ch.eng
            sy2, sf, sg = sy2b[p], sfb[p], sgb[p]
            wtk = wtkb[p]
            dnat = dnatb[p]
            r = ch.run
            # idx first (unblocks ACT wrap asap)
            r(lambda: eng.scalar_tensor_tensor(tD[:, :], sy2[:, 0, :], float(HP), sy2[:, 1, :], OP.mult, OP.add))
            r(lambda: eng.tensor_add(dnat[:, :], tD[:, :], bnat[:, :]), final=(s_dn, p + 1))
            # weight products -> wtk[:, :, k], k order (00, 01, 10, 11)
            r(lambda: eng.tensor_mul(wtk[:, :, 0], sg[:, 0, :], sg[:, 1, :]))
            r(lambda: eng.tensor_mul(wtk[:, :, 1], sg[:, 0, :], sf[:, 1, :]))
            r(lambda: eng.tensor_mul(wtk[:, :, 2], sf[:, 0, :], sg[:, 1, :]))
            r(lambda: eng.tensor_mul(wtk[:, :, 3], sf[:, 0, :], sf[:, 1, :]))

        def emit_fold1(ch, s):
            P = Pb[s % NP_][:, :, :, :]
            A = Ab[s % NA]
            return ch.run(lambda: ch.eng.tensor_add(A[:, :, :, :], P[:, :, :, 0:2], P[:, :, :, 2:4]),
                          final=(s_f1[s], 1))

        @block.vector
        def _(vector: bass.BassEngine):
            ch = Chain(vector, s_cv)
            # zero the wrapped-idx tiles once
            ch.run(lambda: vector.memset(d0[:, :].bitcast(mybir.dt.uint32), 0))
            ch.run(lambda: vector.memset(d1[:, :].bitcast(mybir.dt.uint32), 0))
            vector.wait_ge(s_inx[0], 16)
            vector.wait_ge(s_inb, 16)
            # pair 0: floor+idx entirely on DVE (startup critical path — skips
            # the Pool fracs hop); weights follow after dnat unblocks ACT
            onf0 = onb[0][:, :, :]
            sy2_0, sf0, sg0 = sy2b[0], sfb[0], sgb[0]
            r = ch.run
            r(lambda: vector.tensor_scalar(sy2_0[:, :, :], onf0, -0.5, TWO23, OP.add, OP.add))
            r(lambda: vector.tensor_scalar(sy2_0[:, :, :], sy2_0[:, :, :], -TWO23, 0.0, OP.add, OP.add))
            r(lambda: vector.scalar_tensor_tensor(tD[:, :], sy2_0[:, 0, :], float(HP), sy2_0[:, 1, :], OP.mult, OP.add))
            r(lambda: vector.tensor_add(dnatb[0][:, :], tD[:, :], bnat[:, :]), final=(s_dn, 1))
            r(lambda: vector.tensor_sub(sf0[:, :, :], onf0, sy2_0[:, :, :]))
            r(lambda: vector.tensor_scalar(sg0[:, :, :], sf0[:, :, :], -1.0, 1.0, OP.mult, OP.add))
            r(lambda: vector.tensor_mul(wtkb[0][:, :, 0], sg0[:, 0, :], sg0[:, 1, :]))
            r(lambda: vector.tensor_mul(wtkb[0][:, :, 1], sg0[:, 0, :], sf0[:, 1, :]))
            r(lambda: vector.tensor_mul(wtkb[0][:, :, 2], sf0[:, 0, :], sg0[:, 1, :]))
            r(lambda: vector.tensor_mul(wtkb[0][:, :, 3], sf0[:, 0, :], sf0[:, 1, :]))
            for p in range(1, PAIRS):
                vector.wait_ge(s_frac, p)
                emit_idx_weights(ch, p)
            for s in range(NCHUNK):
                p, c = divmod(s, CH)
                # mul: P[s%2] = G4 * W4
                vector.wait_ge(s_g[s], 16)
                if s >= NP_ and fold1_split[s - NP_] == "g":
                    vector.wait_ge(s_f1[s - NP_], 1)   # P[s%NP_] free
                G4 = Gb[s % NG][:, :, :].bitcast(F16).rearrange("p w (c k) -> p w c k", k=4)
                W4 = wtkb[p][:, c * WCH:(c + 1) * WCH, None, :].broadcast_to([128, WCH, C, 4])
                P = Pb[s % NP_]
                ch.run(lambda G4=G4, W4=W4, P=P: vector.tensor_mul(P[:, :, :, :], G4, W4),
                       final=(s_m[s], 1))
                if fold1_split[s] == "v":
                    if s >= NA:
                        vector.wait_ge(s_f2[s - NA], 1)   # A[s%NA] free
                    if s == NCHUNK - 1:
                        # tail: half-depth drain chain
                        Pl = Pb[s % NP_]
                        Al = Ab[s % NA]
                        h = WCH // 2
                        ch.run(lambda: vector.tensor_add(Al[:, 0:h, :, :], Pl[:, 0:h, :, 0:2], Pl[:, 0:h, :, 2:4]),
                               final=(s_f1[s], 1))
                        ch.run(lambda: vector.tensor_add(Al[:, h:WCH, :, :], Pl[:, h:WCH, :, 0:2], Pl[:, h:WCH, :, 2:4]),
                               final=(s_f1[s], 2))
                    else:
                        emit_fold1(ch, s)

        @block.scalar
        def _(act: bass.BassEngine):
            # rearrange dnat [128h, 128w] -> wrapped d[st] partitions 0-31:
            # d[q, w*8+k] = dnat[q+16k, w]. Pair 0 wraps in two w-halves so
            # the first gathers can start after half 0 (sem s_wr); everything
            # later counts on s_wr2 in 84-inc units.
            def emit_wrap(p, w0, w1, sem):
                st = p % 2
                dnat = dnatb[p]
                dwrap = db[st][:, :].rearrange("p (w k) -> p w k", k=8)
                for k in range(0, 8, 2):   # even k: engine copy (32-aligned src)
                    act.copy(dwrap[0:16, w0:w1, k],
                             dnat[16 * k:16 * (k + 1), w0:w1]).then_inc(sem, 1)
                with nc.allow_non_contiguous_dma(reason="4KB idx-wrap strided dst"):
                    for k in range(1, 8, 2):   # odd k: tiny DMA (no partition align)
                        act.dma_start(dwrap[0:16, w0:w1, k],
                                      dnat[16 * k:16 * (k + 1), w0:w1]).then_inc(sem, 16)

            def emit_repl(p, w0, w1, sem, seen):
                st = p % 2
                act.wait_ge(sem, seen)
                act.dma_start(db[st][16:32, w0 * 8:w1 * 8],
                              db[st][0:16, w0 * 8:w1 * 8]).then_inc(sem, 16)

            act.wait_ge(s_dn, 1)
            emit_wrap(0, 0, 64, s_wr)
            emit_repl(0, 0, 64, s_wr, 68)
            emit_wrap(0, 64, 128, s_wr2)
            emit_repl(0, 64, 128, s_wr2, 68)
            for p in range(1, PAIRS):
                act.wait_ge(s_wr2, 84 * p)   # drain own prior DMA incs
                act.wait_ge(s_dn, p + 1)
                if p >= 2:
                    # d[st] reuse: ALL gathers of pair p-2 must be done
                    for cc in range(CH):
                        act.wait_ge(s_g[CH * (p - 2) + cc], 16)
                emit_wrap(p, 0, 128, s_wr2)
                emit_repl(p, 0, 128, s_wr2, 84 * p + 68)

        @block.gpsimd
        def _(gpsimd: bass.BassGpSimd):
            chg = Chain(gpsimd, s_cg)
            gpsimd.load_library(mlp)

            def frac_final_fix(p):
                # emit_fracs used final=(s_frac, 0) marker; replace with actual
                pass

            def emit_fracs_pool(p):
                gpsimd.wait_ge(s_inx[p], 16)
                onf = onb[p][:, :, :]
                sy2, sf, sg = sy2b[p], sfb[p], sgb[p]
                r = chg.run
                # floor(x) = round_ne(x-0.5) via the fp32 TWO23 trick.
                # Ties (x within ~1ulp of an integer) may floor one down, but
                # bilinear interpolation is continuous there: the weight
                # compensates the index exactly, so the output is unchanged.
                r(lambda: gpsimd.tensor_scalar(sy2[:, :, :], onf, -0.5, TWO23, OP.add, OP.add))
                r(lambda: gpsimd.tensor_scalar(sy2[:, :, :], sy2[:, :, :], -TWO23, 0.0, OP.add, OP.add))
                r(lambda: gpsimd.tensor_sub(sf[:, :, :], onf, sy2[:, :, :]))
                r(lambda: gpsimd.tensor_scalar(sg[:, :, :], sf[:, :, :], -1.0, 1.0, OP.mult, OP.add),
                  final=(s_frac, p))

            def emit_gather(s):
                p, c = divmod(s, CH)
                if p == 0 and c < CH // 2:
                    gpsimd.wait_ge(s_wr, 84)       # pair-0 first w-half wrapped
                else:
                    gpsimd.wait_ge(s_wr2, 84 * (p + 1))
                if s >= NG:
                    gpsimd.wait_ge(s_m[s - NG], 1)   # G[s%NG] free
                ic = NIDX_CH // 16
                gpsimd.dma_gather(
                    Gb[s % NG][:, :, :],
                    patches[p, :, :],
                    db[p % 2][:, c * ic:(c + 1) * ic],
                    NIDX_CH,
                    NIDX_CH,
                    64,
                    single_packet=False,
                ).then_inc(s_g[s], 16)

            def emit_fold2(s):
                A = Ab[s % NA]
                B = Bb[s % NB]
                if s >= NB:
                    gpsimd.wait_ge(s_out[s - NB], 16)   # B[s%NB] free
                if s == NCHUNK - 1:
                    h = WCH // 2
                    gpsimd.wait_ge(s_f1[s], 1)
                    chg.run(lambda: gpsimd.tensor_add(B[:, 0:h, :], A[:, 0:h, :, 0], A[:, 0:h, :, 1]),
                            final=(s_f2[s], 1))
                    gpsimd.wait_ge(s_f1[s], 2)
                    chg.run(lambda: gpsimd.tensor_add(B[:, h:WCH, :], A[:, h:WCH, :, 0], A[:, h:WCH, :, 1]),
                            final=(s_f2[s], 2))
                else:
                    gpsimd.wait_ge(s_f1[s], 1)
                    chg.run(lambda: gpsimd.tensor_add(B[:, :, :], A[:, :, :, 0], A[:, :, :, 1]),
                            final=(s_f2[s], 1))

            def emit_fold1_pool(s):
                gpsimd.wait_ge(s_m[s], 1)
                if s >= NA:
                    gpsimd.wait_ge(s_f2[s - NA], 1)   # A[s%NA] free
                emit_fold1(chg, s)

            # static schedule: fracs interleaved with gathers and folds
            order_env = _os.environ.get("V2_POOL_ORDER")
            lag = int(_os.environ.get("V2_LAG", "2"))
            if order_env:
                order = [tuple(tok.split(":")) for tok in order_env.split(",")]
                order = [(a, int(b)) for a, b in order]
            else:
                order = [("fr", p) for p in range(1, PAIRS)]
                done = 0
                for s in range(NCHUNK):
                    order.append(("g", s))
                    while done <= s - lag:
                        order.append(("f1", done))
                        order.append(("f2", done))
                        done += 1
                while done < NCHUNK:
                    order.append(("f1", done))
                    order.append(("f2", done))
                    done += 1
            for kind, i in order:
                if kind == "fr":
                    emit_fracs_pool(i)
                elif kind == "g":
                    emit_gather(i)
                elif kind == "f1":
                    if fold1_split[i] == "g":
                        emit_fold1_pool(i)
                elif kind == "f2":
                    emit_fold2(i)

    nc.compile()
    return nc


# ---------------- host-side helpers ----------------

def build_patches_all(imgs_pairs):
    """imgs_pairs: (NPAIR, C, H, W) f32 -> (NPAIR, NROWS, 32) u64.

    Row at anchor (hh, ww) = fp16[c][k]: c-major, 4 corners packed per
    channel: k order (0,0), (0,1), (1,0), (1,1)."""
    npair = imgs_pairs.shape[0]
    hw_c = np.ascontiguousarray(np.transpose(imgs_pairs, (0, 2, 3, 1))).astype(np.float16)
    padded = np.zeros((npair, HP + 1, HP + 1, C), np.float16)
    padded[:, PAD:PAD + H, PAD:PAD + W] = hw_c
    P = np.empty((npair, HP, HP, C, 4), np.float16)
    P[:, :, :, :, 0] = padded[:, 0:HP, 0:HP]
    P[:, :, :, :, 1] = padded[:, 0:HP, 1:HP + 1]
    P[:, :, :, :, 2] = padded[:, 1:HP + 1, 0:HP]
    P[:, :, :, :, 3] = padded[:, 1:HP + 1, 1:HP + 1]
    return np.ascontiguousarray(P).reshape(npair, NROWS, 128).view(np.uint32)


def base_natural():
    h = np.arange(H).reshape(H, 1)
    w = np.arange(W).reshape(1, W)
    return ((h + PAD) * HP + (w + PAD)).astype(np.float32)


def make_in_map(imgs_pairs, offp):
    return {
        "patches": build_patches_all(imgs_pairs),
        "offn": np.ascontiguousarray(offp),
        "basen": base_natural(),
    }


# ---------------- public entry point ----------------

N_CORES = 8
PAIRS_TOTAL = 32

LAST_EXEC_TIME_NS = None


def kernel(images, offsets):
    """images (4,8,32,128,128) f32; offsets (4,16,128,128) f32 ->
    (4,8,32,128,128) f32 deformable bilinear sampling, on 8 NeuronCores."""
    import os
    global LAST_EXEC_TIME_NS
    from concourse.bass_utils import run_bass_kernel_spmd

    images = np.ascontiguousarray(np.asarray(images, dtype=np.float32))
    offsets = np.ascontiguousarray(np.asarray(offsets, dtype=np.float32))
    imgs = images.reshape(PAIRS_TOTAL, C, H, W)
    offp = offsets.reshape(4, 8, 2, H, W).reshape(PAIRS_TOTAL, 2, H, W)

    nc = build_nc()
    in_maps = []
    for core in range(N_CORES):
        sl = slice(core * PAIRS, (core + 1) * PAIRS)
        in_maps.append(make_in_map(imgs[sl], offp[sl]))
    trace = bool(os.environ.get("DK_TRACE"))
    res = run_bass_kernel_spmd(nc, in_maps, list(range(N_CORES)), trace=trace)
    if trace:
        LAST_EXEC_TIME_NS = res.exec_time_ns
        if res.instructions_and_trace:
            print("trace path:", res.instructions_and_trace[1])
    outs = [np.asarray(res.results[i]["out"]) for i in range(N_CORES)]
    full = np.concatenate(outs, axis=0).astype(np.float32)   # (32, H, W, C)
    full = np.transpose(full, (0, 3, 1, 2))                  # (32, C, H, W)
    return np.ascontiguousarray(full.reshape(4, 8, C, H, W)).astype(np.float32)
